# revision 40
# baseline (speedup 1.0000x reference)
"""Distributed Trainium2 Bass kernel for a llama-style GQA attention block.

Problem: x[2,2048,4096] -> QKV proj, interleaved RoPE, causal GQA attention
(32 q heads / 8 kv heads), output proj -> out[2,2048,4096], f32.

Strategy: context parallelism over tokens (NOT the tensor-parallel hint: an
output all-reduce of 67 MB would cost as much as all the compute; here the
only collectives are two ~1 MB/rank bf16 all-gathers of K and V).

  - core c => batch b=c//4, in-batch rank j=c%4. Each core owns 512 query
    tokens of its batch: the two 256-token stripes {j, 7-j} (of 8), so the
    causal work is balanced across cores.
  - each core computes Q/K/V projections for its own 512 tokens, applies
    RoPE, all-gathers K and V (bf16) within its 4-core batch group, runs
    attention for all 32 heads over its queries, applies the full output
    projection to its rows, and writes a disjoint slice of the output.

Profile-driven rework of the original context-parallel kernel (the PE here
is GPIO-throttled to ~1.95 GHz, so ~3300 N=512 matmuls set an ~880us floor;
everything else is about never letting the PE or its streams stall):
  - DMA queue roles: wq streams on scalar ONLY with a 6-deep ring (heads
    0-5 prefetched) — both gpsimd (trigger queue) and sync (CC bookkeeper)
    starve while an all-gather is in flight, which cost 20-40us of Q-proj
    stall when wq touched them. Collective input writes go on sync;
    triggers + gathered-K/V readbacks on gpsimd (lenient deadlines).
  - Few mid-size descriptors everywhere: many small ones round-robin the
    shared DMA-completion semaphores and false-serialize across queues;
    single huge ones add latency. x stripes over all 3 queues at startup.
  - Attention: exp runs on [128, 2x512] tile PAIRS (one scalar instruction
    per two key tiles, halving the ~244-cycle/instr overhead; scalar was
    the attention bottleneck at 17.3us/pair vs PE ~13). Tiles are ordered
    by ascending absolute key position so each core's causally-valid set is
    an even-length prefix: a 2-tile group is then uniformly valid or
    invalid per core, expressed as a single per-group exp bias (0/-30000)
    from a host table. Softmax denominators: eager bf16 add-chains on the
    vector engine ([128,1024]-wide adds) + one ones-matmul per slot
    (vs 6 quad matmuls/pair before). The diag group is computed FIRST
    (its mask mult never gates the slot tail) but PV-consumed LAST (the
    mask may lag the vector queue at attention start); each slot's
    denominator/normalize tail is deferred into the next slot so the PE
    never waits on the vector chain.

Numerics: bf16 matmuls with f32 PSUM accumulation. Softmax without max
subtraction (scores are ~N(0,1); exp cannot overflow). The 1/sqrt(128)
score scale is folded into Q's RoPE tables. The interleaved RoPE pairs are
de-interleaved by permuting wq/wk rows on the host (QK^T is invariant to a
shared intra-head permutation), making RoPE contiguous [64, T] vector ops.
"""

import sys

sys.path.insert(0, "/opt/trn_rl_repo")

import numpy as np
import ml_dtypes

import concourse.bass as bass
import concourse.mybir as mybir
import concourse.tile as tile
from concourse import bacc
from concourse.bass_utils import run_bass_kernel_spmd

# problem dims
DIM = 4096
N_HEADS = 32
N_KV_HEADS = 8
HEAD_DIM = 128
BSZ, SEQLEN = 2, 2048

N_CORES = 8
CPB = 4            # cores per batch
T_LOC = 512        # query tokens per core
STRIPE = 256       # 256-token query stripe; 2 per core
P = 128
DM_TILES = DIM // P  # 32
N_GT = 16          # gathered 128-token key tiles per batch

BF16 = mybir.dt.bfloat16
F32 = mybir.dt.float32
NEG = -30000.0

# abs 128-token tile index of each gathered tile position (one batch group):
# in-batch rank r contributes stripes r then 7-r, each two 128-tiles.
A_MAP = []
for _r in range(CPB):
    A_MAP += [2 * _r, 2 * _r + 1, 2 * (7 - _r), 2 * (7 - _r) + 1]
# storage index of absolute 128-tile a in the gathered layout
IDX_OF_ABS = {a: i for i, a in enumerate(A_MAP)}

# per (slot) the gathered tiles any core may need, in ascending abs order,
# grouped in pairs (even-length valid prefixes per core -> per-GROUP bias)
SLOT_ABS = [list(range(6)), list(range(14))]  # slot0: abs<6, slot1: abs<14
SLOT_GROUPS = [
    [(IDX_OF_ABS[a], IDX_OF_ABS[a + 1]) for a in rng[::2]] for rng in SLOT_ABS
]
N_GRP = [len(SLOT_GROUPS[0]) + 1, len(SLOT_GROUPS[1]) + 1]  # + diag group


def build_kernel():
    nc = bacc.Bacc("TRN2", target_bir_lowering=False, debug=False,
                   num_devices=N_CORES)

    # ---- per-core inputs (host-prepped layouts, see _prep_inputs) ----
    # xt is partition-major so multi-tile chunks are single descriptors
    xt_ext = nc.declare_dram_parameter("xt", [P, DM_TILES, T_LOC], BF16,
                                       isOutput=False)
    wqt_ext = nc.declare_dram_parameter("wqt", [32, P, DM_TILES, P], BF16,
                                        isOutput=False)
    wkt_ext = nc.declare_dram_parameter("wkt", [N_KV_HEADS, P, DM_TILES, P],
                                        BF16, isOutput=False)
    wvt_ext = nc.declare_dram_parameter("wvt", [2, 4, P, 8, 512], BF16,
                                        isOutput=False)
    wot_ext = nc.declare_dram_parameter("wot", [8, P, DM_TILES, 512], BF16,
                                        isOutput=False)
    cosq_ext = nc.declare_dram_parameter("cosq", [64, T_LOC], BF16, isOutput=False)
    sinq_ext = nc.declare_dram_parameter("sinq", [64, T_LOC], BF16, isOutput=False)
    cosk_ext = nc.declare_dram_parameter("cosk", [64, T_LOC], BF16, isOutput=False)
    sink_ext = nc.declare_dram_parameter("sink", [64, T_LOC], BF16, isOutput=False)
    # diag masks for the two local key tiles of a slot, side by side
    dmask_ext = nc.declare_dram_parameter("dmask", [P, 2, T_LOC], BF16,
                                          isOutput=False)
    # per-(slot, 2-tile group) exp bias: 0 (valid) or -30000 (invalid)
    bias_ext = nc.declare_dram_parameter("bias", [P, 2, 8], F32, isOutput=False)
    out_ext = nc.declare_dram_parameter("out", [T_LOC, DIM], BF16, isOutput=True)

    groups = [[0, 1, 2, 3], [4, 5, 6, 7]]

    with tile.TileContext(nc) as tc:
        with (
            tc.tile_pool(name="res", bufs=1) as res,
            tc.tile_pool(name="qa", bufs=17) as qa,
            tc.tile_pool(name="vstr", bufs=3) as vstr,
            tc.tile_pool(name="dram", bufs=1, space="DRAM") as dram,
        ):
            # ---------- resident tiles ----------
            ksend = res.tile([P, N_KV_HEADS, T_LOC], BF16)
            # local V, head-major: [tok, g, local tile, 128 v-cols]
            vsend = res.tile([P, N_KV_HEADS, 4, P], BF16)
            cosq = res.tile([64, T_LOC], BF16)
            sinq = res.tile([64, T_LOC], BF16)
            cosk = res.tile([64, T_LOC], BF16)
            sink = res.tile([64, T_LOC], BF16)
            dmask = res.tile([P, 2, T_LOC], BF16)
            bias_sb = res.tile([P, 2, 8], F32)
            ones_col = res.tile([P, 1], BF16)
            dume = res.tile([P, 1], F32)
            nc.vector.memset(ones_col[:], 1.0)
            # preload the Exp activation table off the critical path
            nc.scalar.activation(dume[:], ones_col[:],
                                 mybir.ActivationFunctionType.Exp)

            cc_k_in = dram.tile([P, N_KV_HEADS, T_LOC], BF16)
            cc_k_out = dram.tile([CPB, P, N_KV_HEADS, T_LOC], BF16)
            cc_v_in = dram.tile([P, N_KV_HEADS, 4, P], BF16)
            cc_v_out = dram.tile([CPB, P, N_KV_HEADS, 4, P], BF16)

            # small resident loads early, on the sync queue (it is reserved
            # for collective-adjacent traffic afterwards)
            nc.sync.dma_start(cosk[:], cosk_ext[:])
            nc.sync.dma_start(sink[:], sink_ext[:])
            nc.sync.dma_start(cosq[:], cosq_ext[:])
            nc.sync.dma_start(sinq[:], sinq_ext[:])
            nc.sync.dma_start(dmask[:], dmask_ext[:])
            nc.sync.dma_start(bias_sb[:], bias_ext[:])

            # ---------- phases 1+2: projections + gathers ----------
            with (
                tc.tile_pool(name="p1x", bufs=1) as p1x,
                tc.tile_pool(name="p1w", bufs=1) as p1w,
                tc.tile_pool(name="wqp", bufs=6) as wqp,
                tc.tile_pool(name="rt", bufs=2) as rt,
                tc.tile_pool(name="ps1", bufs=3, space="PSUM") as ps1,
            ):
                # gpsimd queue is dedicated to the K-weight stream through
                # the K projection (1 MB/8.4us saturates a single queue);
                # x stripes per-tile over scalar+sync so the first tiles
                # land within ~2us
                wk_blks = {}

                def _wk_fetch(g, split=1):
                    b = p1w.tile([P, DM_TILES, P], BF16, tag="wk", bufs=3,
                                 name=f"wkg{g}")
                    step = DM_TILES // split
                    for d8 in range(0, DM_TILES, step):
                        nc.gpsimd.dma_start(b[:, d8:d8 + step],
                                            wkt_ext[g, :, d8:d8 + step])
                    return b

                # x striped over all three queues in consumption order
                # (few, mid-size descriptors: many small ones round-robin
                # through shared DMA semaphores and false-serialize across
                # queues; single big ones add 10+us of latency)
                wk0 = p1w.tile([P, DM_TILES, P], BF16, tag="wk", bufs=3,
                               name="wkg0")
                nc.gpsimd.dma_start(wk0[:, 0:8], wkt_ext[0, :, 0:8])
                x_sb = p1x.tile([P, DM_TILES, T_LOC], BF16)
                nc.scalar.dma_start(x_sb[:, 0:6], xt_ext[:, 0:6])
                nc.sync.dma_start(x_sb[:, 12:18], xt_ext[:, 12:18])
                for d8 in range(8, DM_TILES, 8):
                    nc.gpsimd.dma_start(wk0[:, d8:d8 + 8],
                                        wkt_ext[0, :, d8:d8 + 8])
                nc.scalar.dma_start(x_sb[:, 6:12], xt_ext[:, 6:12])
                nc.sync.dma_start(x_sb[:, 18:24], xt_ext[:, 18:24])
                nc.scalar.dma_start(x_sb[:, 24:28], xt_ext[:, 24:28])
                nc.sync.dma_start(x_sb[:, 28:32], xt_ext[:, 28:32])
                wk_blks[0] = wk0
                wk_blks[1] = _wk_fetch(1)
                wk_blks[2] = _wk_fetch(2)

                # Q weights stream on scalar only. gpsimd and sync DMAs
                # both starve while a collective is in flight (gpsimd is
                # the trigger queue, sync does CC bookkeeping), so spreading
                # wq onto them cost 20-40us of Q-proj stall in past revs.
                wq_blks = {}

                def _wq_fetch(h):
                    blk = wqp.tile([P, DM_TILES, P], BF16, tag="wqblk",
                                   name=f"wqb{h}")
                    nc.scalar.dma_start(blk[:], wqt_ext[h])
                    return blk

                # 6-deep ring: heads 0-5 land before the Q projection
                # starts, buffering ~50us of consumption against DMA
                # starvation while the V-gather is in flight
                for h in range(6):
                    wq_blks[h] = _wq_fetch(h)

                wv_blks = {}

                def _wv_fetch(i):
                    vh, grp = divmod(i, 4)
                    b = p1w.tile([P, 8, 512], BF16, tag="wv",
                                 bufs=3, name=f"wvb{vh}{grp}")
                    nc.gpsimd.dma_start(b[:], wvt_ext[vh, grp])
                    return b

                # ---- K projection + rope + gather input writes ----
                for g in range(N_KV_HEADS):
                    wk_g = wk_blks.pop(g)
                    ps_k = ps1.tile([P, T_LOC], F32, tag="pj")
                    for dm in range(DM_TILES):
                        nc.tensor.matmul(ps_k[:], wk_g[:, dm],
                                         x_sb[:, dm],
                                         start=(dm == 0),
                                         stop=(dm == DM_TILES - 1))
                    if g + 3 < N_KV_HEADS:
                        wk_blks[g + 3] = _wk_fetch(g + 3)
                    _rope(nc, rt, ps_k, cosk, sink, ksend[:, g])
                    nc.sync.dma_start(cc_k_in[:, g], ksend[:, g])

                # first V-weight blocks: issued after the K-weight stream
                # but before the gather trigger (which would block them)
                for i in range(3):
                    wv_blks[i] = _wv_fetch(i)

                nc.gpsimd.collective_compute(
                    "AllGather", mybir.AluOpType.bypass,
                    replica_groups=groups,
                    ins=[cc_k_in[:]], outs=[cc_k_out[:]])

                # ---- V projection ----
                for vh in range(2):
                    ps_v = [
                        ps1.tile([P, 512], F32, tag=f"pvt{tt}", bufs=1,
                                 name=f"psv{vh}{tt}")
                        for tt in range(4)
                    ]
                    for grp in range(4):
                        blk_i = vh * 4 + grp
                        wv_b = wv_blks.pop(blk_i)
                        for d8 in range(8):
                            dm = grp * 8 + d8
                            for tt in range(4):
                                nc.tensor.matmul(
                                    ps_v[tt][:],
                                    x_sb[:, dm, tt * P:(tt + 1) * P],
                                    wv_b[:, d8], start=(dm == 0),
                                    stop=(dm == DM_TILES - 1))
                        if blk_i + 3 < 8:
                            wv_blks[blk_i + 3] = _wv_fetch(blk_i + 3)
                    for tt in range(4):
                        # ps_v[tt] = [128 tok, 512 v-cols] = heads
                        # vh*4..vh*4+3; scatter into head-major vsend
                        nc.scalar.copy(
                            vsend[:, vh * 4:(vh + 1) * 4, tt, :],
                            ps_v[tt][:])
                nc.sync.dma_start(cc_v_in[:], vsend[:])

                nc.gpsimd.collective_compute(
                    "AllGather", mybir.AluOpType.bypass,
                    replica_groups=groups,
                    ins=[cc_v_in[:]], outs=[cc_v_out[:]])

                # ---- Q projection + rope (overlaps the gathers) ----
                # Roped Q of a head PAIR is stored interleaved as
                # [128, slot, rel_head, 256] so attention can consume both
                # heads of a pair with single N=512 matmuls.
                qt = []
                for pair in range(16):
                    qp_t = qa.tile([P, 2, 2, STRIPE], BF16, tag="qt",
                                   name=f"qp{pair}")
                    for qh_rel in range(2):
                        h = 2 * pair + qh_rel
                        wq_blk = wq_blks.pop(h)
                        ps_q = ps1.tile([P, T_LOC], F32, tag="pj")
                        for dm in range(DM_TILES):
                            nc.tensor.matmul(
                                ps_q[:], wq_blk[:, dm], x_sb[:, dm],
                                start=(dm == 0), stop=(dm == DM_TILES - 1))
                        # refetch into h's ring slot only after h's matmuls
                        # are issued (compile-time WAR then orders the DMA
                        # behind them; the scalar queue has nothing else)
                        if h + 6 < N_HEADS:
                            wq_blks[h + 6] = _wq_fetch(h + 6)
                        _rope(nc, rt, ps_q, cosq, sinq,
                              qp_t[:, :, qh_rel, :])
                    qt.append(qp_t)

            # ---------- phase 3: attention ----------
            # kvp opens after the phase-1 pools close, reusing x_sb's SBUF
            # for the gathered K (read back right after the all-gather; the
            # descriptors still queue on gpsimd behind the V trigger)
            with (
                tc.tile_pool(name="kvp", bufs=1) as kvp,
                tc.tile_pool(name="p5w", bufs=5) as p5w,
            ):
                kfull = kvp.tile([P, N_KV_HEADS, CPB * T_LOC], BF16)
                for r in range(CPB):
                    nc.gpsimd.dma_start(
                        kfull[:, :, r * T_LOC:(r + 1) * T_LOC], cc_k_out[r])
                # wo chunk prefetcher: DMAs issue on the (idle) sync queue
                # during attention; WAR on the 5-buffer ring self-regulates.
                wo_state = {"next": 0, "blks": {}}

                def _wo_fetch_upto(k):
                    while wo_state["next"] < min(k, 64):
                        i = wo_state["next"]
                        ot, afb = divmod(i, 8)
                        t = p5w.tile([P, 4, 512], BF16, tag="woc",
                                     name=f"wo{ot}_{afb}")
                        nc.sync.dma_start(
                            t[:], wot_ext[ot, :, afb * 4:afb * 4 + 4])
                        wo_state["blks"][(ot, afb)] = t
                        wo_state["next"] += 1

                # gathered-V per-head streamer: head g lands in a 3-buffer
                # ring; fetched 2 heads ahead of consumption (pair 2g).
                vs_state = {"next": 0, "blks": {}}

                def _v_fetch_upto(k):
                    while vs_state["next"] < min(k, N_KV_HEADS):
                        g2 = vs_state["next"]
                        t = vstr.tile([P, N_GT, P], BF16, tag="vs",
                                      name=f"vs{g2}")
                        for r in range(CPB):
                            nc.gpsimd.dma_start(t[:, 4 * r:4 * r + 4, :],
                                                cc_v_out[r, :, g2, :, :])
                        vs_state["blks"][g2] = t
                        vs_state["next"] += 1

                with (
                    tc.tile_pool(name="at", bufs=4) as at,
                    tc.tile_pool(name="ps_sc", bufs=2, space="PSUM") as ps_sc,
                    tc.tile_pool(name="ps_pv", bufs=2, space="PSUM") as ps_pv,
                    tc.tile_pool(name="ps_dn", bufs=2, space="PSUM") as ps_dn,
                ):
                    LEADG = 2  # score groups issued ahead of PV consumption
                    _v_fetch_upto(2)
                    attn = []
                    # each slot's denominator/normalize tail is deferred
                    # into the NEXT slot (flushed after its first two score
                    # groups) so the PE never waits on the vector chain
                    pending_tail = [None]

                    def _flush_tail():
                        if pending_tail[0] is not None:
                            pending_tail[0]()
                            pending_tail[0] = None

                    for hp in range(16):
                        g = hp // 2
                        vfull_g = vs_state["blks"][g]
                        _v_fetch_upto(g + 3)
                        a_p = qa.tile([P, 2, 2, STRIPE], BF16, tag="qt",
                                      name=f"attnp{hp}")
                        attn.append(a_p)
                        _wo_fetch_upto(2 * hp)
                        for s in range(2):
                            q_ap = qt[hp][:, s]  # [128, 2, 256] = both heads
                            ps_o = ps_pv.tile([P, T_LOC], F32, tag="pv")
                            ps_d = ps_dn.tile([1, T_LOC], F32, tag="dn")
                            grps = SLOT_GROUPS[s]       # gathered 2-tile grps
                            ng = len(grps) + 1          # diag group first
                            pend = []      # gathered groups awaiting PV
                            diag_sv = None  # diag group: PV'd last (its
                            # mask mult may lag the vector queue early on)
                            npv = 0
                            chain = None  # running bf16 denominator sum
                            for gi in range(ng + LEADG):
                                if gi == 2:
                                    _flush_tail()
                                if gi < ng:
                                    ps_g = ps_sc.tile([P, 2, T_LOC], F32,
                                                      tag="sc")
                                    e_g = at.tile([P, 2, T_LOC], BF16,
                                                  tag="exp", bufs=10,
                                                  name=f"e{hp}{s}{gi}")
                                    if gi == 0:
                                        # diag group first: its mask mult
                                        # never gates the slot tail
                                        for half in range(2):
                                            lt = 2 * s + half
                                            nc.tensor.matmul(
                                                ps_g[:, half],
                                                ksend[:, g,
                                                      lt * P:(lt + 1) * P],
                                                q_ap, start=True, stop=True)
                                        nc.scalar.activation(
                                            e_g[:], ps_g[:],
                                            mybir.ActivationFunctionType.Exp)
                                        e_m = at.tile([P, 2, T_LOC], BF16,
                                                      tag="exp", bufs=10,
                                                      name=f"em{hp}{s}")
                                        nc.vector.tensor_tensor(
                                            e_m[:], e_g[:], dmask[:],
                                            mybir.AluOpType.mult)
                                        e_g = e_m
                                        v_aps = [vsend[:, g, 2 * s, :],
                                                 vsend[:, g, 2 * s + 1, :]]
                                    else:
                                        ia, ib = grps[gi - 1]
                                        for half, idx in ((0, ia), (1, ib)):
                                            nc.tensor.matmul(
                                                ps_g[:, half],
                                                kfull[:, g,
                                                      idx * P:(idx + 1) * P],
                                                q_ap, start=True, stop=True)
                                        nc.scalar.activation(
                                            e_g[:], ps_g[:],
                                            mybir.ActivationFunctionType.Exp,
                                            bias=bias_sb[:, s, gi - 1:gi])
                                        v_aps = [
                                            vfull_g[:, ia, :],
                                            vfull_g[:, ib, :],
                                        ]
                                    if gi == 0:
                                        diag_sv = (e_g, v_aps)
                                    else:
                                        pend.append((e_g, v_aps))
                                    # eager denominator chain ([128,1024]
                                    # bf16 adds on the vector engine)
                                    if gi >= 1:
                                        c = at.tile([P, 2, T_LOC], BF16,
                                                    tag="dq", bufs=3,
                                                    name=f"dq{hp}{s}{gi}")
                                        prev = diag_sv[0] if gi == 1 \
                                            else chain
                                        nc.vector.tensor_tensor(
                                            c[:], prev[:], e_g[:],
                                            mybir.AluOpType.add)
                                        chain = c
                                if gi >= LEADG and pend:
                                    e_j, v_j = pend.pop(0)
                                    for half in range(2):
                                        nc.tensor.matmul(
                                            ps_o[:], v_j[half],
                                            e_j[:, half],
                                            start=(npv == 0),
                                            stop=False)
                                        npv += 1
                            # diag PV last: by now its mask mult is done
                            e_j, v_j = diag_sv
                            for half in range(2):
                                nc.tensor.matmul(
                                    ps_o[:], v_j[half], e_j[:, half],
                                    start=False, stop=(half == 1))
                            def _tail(chain=chain, ps_o=ps_o, ps_d=ps_d,
                                      a_p=a_p, s=s, hp=hp):
                                tfold = at.tile([P, T_LOC], BF16, tag="df",
                                                bufs=2, name=f"df{hp}{s}")
                                nc.vector.tensor_tensor(
                                    tfold[:], chain[:, 0], chain[:, 1],
                                    mybir.AluOpType.add)
                                nc.tensor.matmul(ps_d[:], ones_col[:],
                                                 tfold[:],
                                                 start=True, stop=True)
                                rec1 = at.tile([1, T_LOC], F32, tag="rc1",
                                               bufs=2)
                                nc.vector.reciprocal_approx_fast(rec1[:],
                                                                 ps_d[:])
                                rec_b = at.tile([P, T_LOC], F32, tag="rcb",
                                                bufs=2)
                                nc.gpsimd.partition_broadcast(rec_b[:],
                                                              rec1[:])
                                nc.vector.tensor_tensor(
                                    a_p[:, s], ps_o[:], rec_b[:],
                                    mybir.AluOpType.mult)

                            _flush_tail()
                            pending_tail[0] = _tail
                    _flush_tail()

                # ---------- phase 4: output projection ----------
                with (
                    tc.tile_pool(name="p5s", bufs=4) as p5s,
                    tc.tile_pool(name="ps5", bufs=1, space="PSUM") as ps5,
                ):
                    out_q = [nc.gpsimd, nc.scalar, nc.sync]
                    for ot in range(8):
                        ps_os = [
                            ps5.tile([P, 512], F32, tag=f"po{t4}", bufs=2,
                                     name=f"pso{ot}{t4}")
                            for t4 in range(4)
                        ]
                        for afb in range(8):
                            _wo_fetch_upto(ot * 8 + afb + 4)
                            wo_c = wo_state["blks"].pop((ot, afb))
                            for af_rel in range(4):
                                af = afb * 4 + af_rel
                                for t4 in range(4):
                                    nc.tensor.matmul(
                                        ps_os[t4][:],
                                        attn[af // 2][:, t4 // 2, af % 2,
                                                      (t4 % 2) * P:
                                                      (t4 % 2 + 1) * P],
                                        wo_c[:, af_rel],
                                        start=(af == 0),
                                        stop=(af == DM_TILES - 1))
                        for t4 in range(4):
                            o_st = p5s.tile([P, 512], BF16, tag="ostage",
                                            name=f"ost{ot}{t4}")
                            if t4 % 2 == 0:
                                nc.scalar.copy(o_st[:], ps_os[t4][:])
                            else:
                                nc.vector.tensor_copy(o_st[:], ps_os[t4][:])
                            out_q[(ot * 4 + t4) % 3].dma_start(
                                out_ext[t4 * P:(t4 + 1) * P,
                                        ot * 512:(ot + 1) * 512], o_st[:])

    nc.finalize()
    return nc


def _rope(nc, pool, ps, cos, sin, out_sb):
    """RoPE on de-interleaved layout.

    ps: [128, T] f32 psum; partitions 0:64 = even dims (a), 64:128 = odd (b).
    out[0:64] = a*cos - b*sin; out[64:128] = a*sin + b*cos.
    """
    T = ps.shape[-1]
    a = ps[0:64]
    b = ps[64:128]
    t0 = pool.tile([64, T], F32, tag="ropet0")
    t1 = pool.tile([64, T], F32, tag="ropet1", bufs=1)
    nc.vector.tensor_tensor(t0[:], a, cos[:], mybir.AluOpType.mult)
    nc.vector.tensor_tensor(t1[:], b, sin[:], mybir.AluOpType.mult)
    nc.vector.tensor_tensor(out_sb[0:64], t0[:], t1[:],
                            mybir.AluOpType.subtract)
    nc.vector.tensor_tensor(t0[:], a, sin[:], mybir.AluOpType.mult)
    nc.vector.tensor_tensor(t1[:], b, cos[:], mybir.AluOpType.mult)
    nc.vector.tensor_tensor(out_sb[64:128], t0[:], t1[:], mybir.AluOpType.add)


# ---------------------------------------------------------------------------
# host side
# ---------------------------------------------------------------------------

def _deint_perm(n_heads):
    """Row permutation de-interleaving rope pairs within each head."""
    idx = []
    for h in range(n_heads):
        base = h * HEAD_DIM
        idx += [base + d for d in range(0, HEAD_DIM, 2)]
        idx += [base + d for d in range(1, HEAD_DIM, 2)]
    return np.array(idx)


def _tokens_of_core(c):
    j = c % CPB
    s1, s2 = j, 7 - j
    return np.concatenate([
        np.arange(s1 * STRIPE, (s1 + 1) * STRIPE),
        np.arange(s2 * STRIPE, (s2 + 1) * STRIPE)])


def _prep_inputs(x, wq, wk, wv, wo, freqs_cos, freqs_sin):
    bf16 = ml_dtypes.bfloat16
    f32 = np.float32

    wq_p = wq[_deint_perm(N_HEADS)]
    wk_p = wk[_deint_perm(N_KV_HEADS)]

    # shared blocked weights
    wqt = np.ascontiguousarray(
        wq_p.T.reshape(DM_TILES, P, N_HEADS, P).transpose(2, 1, 0, 3)
    ).astype(bf16)
    wkt = np.ascontiguousarray(
        wk_p.T.reshape(DM_TILES, P, N_KV_HEADS, P).transpose(2, 1, 0, 3)
    ).astype(bf16)
    wvt = np.ascontiguousarray(
        wv.T.reshape(4, 8, P, 2, 512).transpose(3, 0, 2, 1, 4)).astype(bf16)
    wot = np.ascontiguousarray(
        wo.T.reshape(DM_TILES, P, 8, 512).transpose(2, 1, 0, 3)).astype(bf16)

    inv = np.float32(1.0 / np.sqrt(HEAD_DIM))
    cosT = freqs_cos.T.astype(f32)  # [64, S]
    sinT = freqs_sin.T.astype(f32)

    # binary post-exp masks for the two diagonal key tiles (key partition
    # t vs query column q within the 256-stripe; duplicated for both heads)
    t_idx = np.arange(P)[:, None]
    q_idx = np.arange(STRIPE)[None, :]
    mask1 = np.where(t_idx <= q_idx, 1.0, 0.0).astype(f32)
    mask2 = np.where(t_idx + P <= q_idx, 1.0, 0.0).astype(f32)
    mask1 = np.concatenate([mask1, mask1], axis=1)  # both heads
    mask2 = np.concatenate([mask2, mask2], axis=1)
    dmask = np.stack([mask1, mask2], axis=1).astype(bf16)  # [P, 2, 512]

    in_maps = []
    for c in range(N_CORES):
        b, j = c // CPB, c % CPB
        tok = _tokens_of_core(c)
        xt = np.ascontiguousarray(
            x[b][tok].T.reshape(DM_TILES, P, T_LOC).transpose(1, 0, 2)
        ).astype(bf16)
        # per-(slot, group) exp bias: group m of slot s is valid iff its
        # two tiles are within the core's causally-valid prefix
        bias = np.zeros((P, 2, 8), f32)
        for s in range(2):
            s_abs = j if s == 0 else 7 - j
            for m, (ia, ib) in enumerate(SLOT_GROUPS[s]):
                if A_MAP[ib] >= 2 * s_abs:
                    bias[:, s, m] = NEG
        in_maps.append({
            "xt": xt,
            "wqt": wqt, "wkt": wkt, "wvt": wvt, "wot": wot,
            "cosq": np.ascontiguousarray(cosT[:, tok] * inv).astype(bf16),
            "sinq": np.ascontiguousarray(sinT[:, tok] * inv).astype(bf16),
            "cosk": np.ascontiguousarray(cosT[:, tok]).astype(bf16),
            "sink": np.ascontiguousarray(sinT[:, tok]).astype(bf16),
            "dmask": dmask,
            "bias": bias,
        })
    return in_maps


_NC_CACHE = None


def _get_nc():
    global _NC_CACHE
    if _NC_CACHE is None:
        _NC_CACHE = build_kernel()
    return _NC_CACHE


def kernel(x, wq, wk, wv, wo, freqs_cos, freqs_sin, _trace=False):
    x = np.asarray(x, dtype=np.float32)
    in_maps = _prep_inputs(
        x, np.asarray(wq, np.float32), np.asarray(wk, np.float32),
        np.asarray(wv, np.float32), np.asarray(wo, np.float32),
        np.asarray(freqs_cos, np.float32), np.asarray(freqs_sin, np.float32))
    nc = _get_nc()
    res = run_bass_kernel_spmd(nc, in_maps, core_ids=list(range(N_CORES)),
                               trace=_trace)
    out = np.empty((BSZ, SEQLEN, DIM), np.float32)
    for c in range(N_CORES):
        out[c // CPB, _tokens_of_core(c)] = \
            np.asarray(res.results[c]["out"]).astype(np.float32)
    if _trace:
        kernel.last_exec_time_ns = res.exec_time_ns
        kernel.last_results = res
    return out


if __name__ == "__main__":
    build_kernel()
    print("built ok")
